# revision 4
# baseline (speedup 1.0000x reference)
"""Masked self-attention (softmax over axis=1) Bass kernel for TRN2, 8 cores.

Reference semantics (per batch b):
    attn[l, m] = <a_l, a_m> * temperature            [L, L]
    attn = where(mask[l, m], attn, -1e7)
    P = softmax(attn, axis=l)                        (softmax over dim 0)
    out[m, :] = sum_l P[l, m] * a[l, :]              [L, H]

v2 design (pure data parallel, 4 batches/core, no collectives):

  Staging per batch (chunked, 3 DMA paths in parallel):
    t2 bf16 [128, 8, H+1] <- gpsimd SWDGE cast-DMA straight from f32 a
        (SWDGE casts inline; kills the f32 SBUF staging + DVE cast)
    scratch DRAM bf16 <- bounce t2 out (scalar ring)
    AT bf16 [128, 6, L] <- xbar transpose DMAs (sync ring ONLY - mode
        switches serialize a ring), chunked so S can start early
    AT8 fp8e4 <- DVE cast of AT chunks
  S = AT8^T @ AT8 in fp8 DoubleRow (2 d-tiles per pass, 3 passes,
      2x PE throughput; rel-err budget checked in numpy: ~3e-3)
  S' = mask_u8 * (BIG/temp) + S    (DVE scalar_tensor_tensor)
  E = exp(temp*S' - BIG) -> bf16   (ACT)
  [feat | denom] = E^T @ [t2 | 1]  (PE bf16, paired-chunk weight reuse)
  out = feat * (1/denom)           (DVE reciprocal + ACT scale-copy),
      stores alternate scalar/gpsimd rings to kill the tail.
  PE warmup: ~16 dummy matmuls during batch-0 staging flip the HAM
      clock gate to 8/8 before real work lands.
"""

import os as _os
import sys

import numpy as np

sys.path.insert(0, "/opt/trn_rl_repo")

B, L, H = 32, 1024, 768
N_CORES = 8
B_LOCAL = B // N_CORES  # 4 batches per core
LT = L // 128  # 8 l-tiles
DT = H // 128  # 6 d-tiles
DP = DT // 2  # 3 d-tile pairs (DoubleRow)
BIG = 50.0

N_TRANSPOSE = int(_os.environ.get("K_NT", "2"))  # xbar chunks per batch
N_STAGE = int(_os.environ.get("K_NST", "4"))  # t2 cast-load chunks
WARM = int(_os.environ.get("K_WARM", "1"))  # PE warmup matmuls
OUT_SPREAD = int(_os.environ.get("K_OUTS", "1"))  # alternate out rings

_CACHE = {}


def _build(temp: float, repeats: int = 1, bench: bool = False):
    from contextlib import ExitStack

    import concourse.mybir as mybir
    from concourse import bacc, tile

    f32 = mybir.dt.float32
    bf16 = mybir.dt.bfloat16
    fp8 = mybir.dt.float8e4
    u8 = mybir.dt.uint8
    DR = mybir.MatmulPerfMode.DoubleRow

    nc = bacc.Bacc(
        "TRN2", target_bir_lowering=False, debug=False, num_devices=N_CORES
    )

    if bench:
        # Timing-only variant: big tensors live in Internal DRAM (content
        # irrelevant - instruction stream is identical), so per-call axon
        # transfer overhead stays tiny and the R-repeat delta is clean.
        nc.dram_tensor("bench_in", [1, 4], f32, kind="ExternalInput")
        nc.dram_tensor("out", [1, 4], f32, kind="ExternalOutput")
        a_ext = nc.dram_tensor("a", [B_LOCAL, L, H], f32).ap()
        m_ext = nc.dram_tensor("mask_a", [B_LOCAL, L, L], u8).ap()
        out_ext = nc.dram_tensor("out_int", [B_LOCAL, L, H], f32).ap()
    else:
        a_ext = nc.dram_tensor("a", [B_LOCAL, L, H], f32, kind="ExternalInput").ap()
        m_ext = nc.dram_tensor(
            "mask_a", [B_LOCAL, L, L], u8, kind="ExternalInput"
        ).ap()
        out_ext = nc.dram_tensor(
            "out", [B_LOCAL, L, H], f32, kind="ExternalOutput"
        ).ap()

    big_over_temp = BIG / temp

    with tile.TileContext(nc) as tc, ExitStack() as ctx:
        t2_pool = ctx.enter_context(tc.tile_pool(name="t2", bufs=2))
        at_pool = ctx.enter_context(tc.tile_pool(name="at", bufs=2))
        at8_pool = ctx.enter_context(tc.tile_pool(name="at8", bufs=2))
        mask_pool = ctx.enter_context(tc.tile_pool(name="mask", bufs=2))
        e_pool = ctx.enter_context(tc.tile_pool(name="e", bufs=2))
        sp_pool = ctx.enter_context(tc.tile_pool(name="sp", bufs=4))
        out_pool = ctx.enter_context(tc.tile_pool(name="outp", bufs=3))
        rc_pool = ctx.enter_context(tc.tile_pool(name="rc", bufs=3))
        dram_pool = ctx.enter_context(
            tc.tile_pool(name="bounce", bufs=2, space="DRAM")
        )
        psum_s = ctx.enter_context(tc.tile_pool(name="ps_s", bufs=2, space="PSUM"))
        psum_o = ctx.enter_context(tc.tile_pool(name="ps_o", bufs=2, space="PSUM"))
        const_pool = ctx.enter_context(tc.tile_pool(name="const", bufs=1))

        neg_big = const_pool.tile([128, 1], f32)
        nc.vector.memset(neg_big[:], -BIG)

        if WARM:
            wz = const_pool.tile([128, 512], bf16)
            nc.vector.memset(wz[:], 0.0)

        for bi, b in enumerate(
            [b for _ in range(repeats) for b in range(B_LOCAL)]
        ):
            a_v = a_ext[b].rearrange("(i p) d -> p i d", p=128)  # [128, 8, 768]
            m_v = m_ext[b].rearrange("(i p) m -> p i m", p=128)  # [128, 8, 1024]
            o_v = out_ext[b].rearrange("(i p) d -> p i d", p=128)

            t2 = t2_pool.tile([128, LT, H + 1], bf16)
            scratch = dram_pool.tile([L, H], bf16)
            s_v = scratch[:].rearrange("(i p) d -> p i d", p=128)
            at = at_pool.tile([128, DT, L], bf16)
            at8 = at8_pool.tile([128, DT, L], fp8)
            msk = mask_pool.tile([128, LT, L], u8)

            # 1. cast-load t2 (gpsimd SWDGE casts f32->bf16 inline)
            lchunk = LT // N_STAGE
            for ci in range(N_STAGE):
                sl = slice(lchunk * ci, lchunk * (ci + 1))
                nc.gpsimd.dma_start(out=t2[:, sl, 0:H], in_=a_v[:, sl, :])
            nc.vector.memset(t2[:, :, H : H + 1], 1.0)
            # 2. bounce-out to DRAM bf16 (scalar ring)
            for ci in range(N_STAGE):
                sl = slice(lchunk * ci, lchunk * (ci + 1))
                nc.scalar.dma_start(out=s_v[:, sl, :], in_=t2[:, sl, 0:H])
            # 3. xbar transposes (sync ring ONLY)
            rt = L // N_TRANSPOSE
            for ti in range(N_TRANSPOSE):
                rows = slice(rt * ti, rt * (ti + 1))
                nc.sync.dma_start(
                    out=at[:, :, rows], in_=scratch[rows, :], transpose=True
                )
            # 4. cast AT -> fp8 per xbar chunk (vector)
            for ti in range(N_TRANSPOSE):
                rows = slice(rt * ti, rt * (ti + 1))
                nc.vector.tensor_copy(at8[:, :, rows], at[:, :, rows])
            # 5. mask load (scalar ring), 2 chunks
            for ci in range(2):
                sl = slice(4 * ci, 4 * (ci + 1))
                nc.scalar.dma_start(out=msk[:, sl, :], in_=m_v[:, sl, :])

            # S rows: fp8 DoubleRow, 3 d-pair passes per 512-col chunk.
            # Batch 0 runs column-chunk-major so the left half starts as
            # soon as xbar chunk 0 lands; later batches run jp-major and
            # reuse loaded weights across the two column chunks.
            e = e_pool.tile([128, LT, L], bf16)
            for li in range(LT):
                ps = psum_s.tile([128, L], f32)
                lh = slice(128 * li, 128 * (li + 1))
                if WARM and bi == 0 and li == 0:
                    # Dummy matmuls during batch-0 staging: trip the PE HAM
                    # clock gate to 8/8 (~3.4us of activity) before the real
                    # S lands. They only depend on wz, so the PE runs them
                    # immediately; the real start=True group overwrites.
                    for wi in range(16):
                        nc.tensor.matmul(
                            ps[:, 0:512],
                            lhsT=wz[:, 0:128],
                            rhs=wz[:],
                            start=True,
                            stop=True,
                            skip_group_check=True,
                        )
                if bi == 0:
                    for c in range(2):
                        for jp in range(DP):
                            nc.tensor.matmul(
                                ps[:, 512 * c : 512 * (c + 1)],
                                lhsT=at8[:, 2 * jp : 2 * jp + 2, lh],
                                rhs=at8[:, 2 * jp : 2 * jp + 2, 512 * c : 512 * (c + 1)],
                                start=(jp == 0),
                                stop=(jp == DP - 1),
                                perf_mode=DR,
                            )
                else:
                    for jp in range(DP):
                        for c in range(2):
                            mm = nc.tensor.matmul(
                                ps[:, 512 * c : 512 * (c + 1)],
                                lhsT=at8[:, 2 * jp : 2 * jp + 2, lh],
                                rhs=at8[:, 2 * jp : 2 * jp + 2, 512 * c : 512 * (c + 1)],
                                start=(jp == 0),
                                stop=(jp == DP - 1),
                                perf_mode=DR,
                            )
                            if c == 1:
                                mm.ins.ldweights = False
                sp = sp_pool.tile([128, L], f32)
                nc.vector.scalar_tensor_tensor(
                    out=sp[:],
                    in0=msk[:, li, :],
                    scalar=big_over_temp,
                    in1=ps[:],
                    op0=mybir.AluOpType.mult,
                    op1=mybir.AluOpType.add,
                )
                nc.scalar.activation(
                    out=e[:, li, :],
                    in_=sp[:],
                    func=mybir.ActivationFunctionType.Exp,
                    bias=neg_big[:],
                    scale=temp,
                )

            # [feat | denom] = E^T @ [t2 | 1]; normalize; store.
            for mi in range(LT):
                po = psum_o.tile([128, H + 1], f32)
                for li in range(LT):
                    w = e[:, li, 128 * mi : 128 * (mi + 1)]
                    nc.tensor.matmul(
                        po[:, 0:512],
                        lhsT=w,
                        rhs=t2[:, li, 0:512],
                        start=(li == 0),
                        stop=(li == LT - 1),
                    )
                    mm2nd = nc.tensor.matmul(
                        po[:, 512 : H + 1],
                        lhsT=w,
                        rhs=t2[:, li, 512 : H + 1],
                        start=(li == 0),
                        stop=(li == LT - 1),
                    )
                    mm2nd.ins.ldweights = False
                rc = rc_pool.tile([128, 1], f32)
                nc.vector.reciprocal(rc[:], po[:, H : H + 1])
                ot = out_pool.tile([128, H], f32)
                nc.scalar.activation(
                    out=ot[:],
                    in_=po[:, 0:H],
                    func=mybir.ActivationFunctionType.Copy,
                    scale=rc[:],
                )
                if OUT_SPREAD:
                    out_eng = nc.gpsimd if mi % 2 == 0 else nc.scalar
                else:
                    out_eng = nc.gpsimd
                out_eng.dma_start(out=o_v[:, mi, :], in_=ot[:])

    nc.compile()
    return nc


def _get_nc(temp: float, repeats: int = 1, bench: bool = False):
    key = (round(float(temp), 12), repeats, bench)
    if key not in _CACHE:
        _CACHE[key] = _build(float(temp), repeats, bench)
    return _CACHE[key]


def run(a, mask_a, temperature=None, trace=False):
    from concourse.bass_utils import run_bass_kernel_spmd

    a = np.ascontiguousarray(np.asarray(a, dtype=np.float32))
    mask_u8 = np.ascontiguousarray(np.asarray(mask_a)).view(np.uint8)
    if temperature is None:
        temperature = 1.0 / np.sqrt(np.float32(H))
    temp = float(np.asarray(temperature, dtype=np.float32))

    nc = _get_nc(temp)
    in_maps = [
        {
            "a": a[c * B_LOCAL : (c + 1) * B_LOCAL],
            "mask_a": mask_u8[c * B_LOCAL : (c + 1) * B_LOCAL],
        }
        for c in range(N_CORES)
    ]
    res = run_bass_kernel_spmd(
        nc, in_maps, core_ids=list(range(N_CORES)), trace=trace
    )
    out = np.concatenate([res.results[c]["out"] for c in range(N_CORES)], axis=0)
    return out, res


def kernel(a, mask_a, temperature=None, **_):
    out, _res = run(a, mask_a, temperature)
    return out
